# revision 17
# baseline (speedup 1.0000x reference)
"""3-layer GAT (DGL-style GATConv) on one TRN2 chip (8 NeuronCores).

Sharding: nodes are range-partitioned across the 8 cores (graph parallel).
Edges are bucketed by destination shard and sorted by destination; each core
owns the edge softmax + message aggregation for its node range.  Per layer,
each core computes its slice of the packed projection [feat | el] (el
attention dot products folded into the weight matrix on the host), the
slices are AllGather'ed, and per-edge source records are fetched from the
gathered table with batched SWDGE dma_gather (two calls per destination
tile: the table is split at the rank-4 boundary so row indices fit int16).
Scatter-add into destinations is a matmul with a 0/1 selector matrix built
from an iota/is_equal compare.
"""

import os
import sys

import numpy as np

if "/opt/trn_rl_repo" not in sys.path:
    sys.path.insert(0, "/opt/trn_rl_repo")

import ml_dtypes

P = 128            # partitions / block size
NSH = 8            # shards (NeuronCores)
REC = 384          # record width in bf16 units: 256 feat + 8 el-f32 + pad
RECF = REC // 2    # record width in f32 units
ELF = 128          # f32-unit offset of el inside a record

# problem constants
N, E = 50000, 800000
IN_DIM, HID, HEADS, OUT_DIM = 512, 256, 4, 256
NEG_SLOPE = 0.2
NS = N // NSH                      # 6250 real nodes per shard
TILES = (NS + P - 1) // P          # 49
NSP = TILES * P                    # 6272 padded nodes per shard
HLOC = 3200                        # local-row split: half-table AllGathers
HTA = NSH * HLOC                   # 25600 rows (fits int16)
HTB = NSH * (NSP - HLOC)           # 24576 rows


def preprocess_edges(src, dst, ns, nsp, nsh):
    """Bucket edges by dst shard, sort by dst, tile into 128-node dst tiles,
    then split each tile's edges by source rank-group (ranks 0-3 vs 4-7) and
    block into 128-edge blocks.  Block counts are maxed across shards so all
    cores share one instruction stream.

    Returns (nbA, nbB, baseA, baseB, base, B, and per-core idxA/idxB int16
    wrapped index arrays, dstloc [P, B], dstloc_row [B*P]).
    """
    tiles = nsp // P
    src = np.asarray(src).astype(np.int64)
    dst = np.asarray(dst).astype(np.int64)
    shard = dst // ns
    local = dst - shard * ns
    s_sh = src // ns
    u_src = src - s_sh * ns
    half = (u_src >= HLOC).astype(np.int64)
    prow = np.where(half == 0, s_sh * HLOC + u_src,
                    s_sh * (nsp - HLOC) + (u_src - HLOC)).astype(np.int64)

    cnt = np.zeros((nsh, tiles, 2), np.int64)
    np.add.at(cnt, (shard, local // P, half), 1)
    nbh = (-(-cnt // P)).max(axis=0)           # [tiles, 2]
    for t in range(tiles):
        if nbh[t].sum() == 0:
            nbh[t, 0] = 1
    nbA, nbB = nbh[:, 0].copy(), nbh[:, 1].copy()
    nb = nbA + nbB
    BA, BB = int(nbA.sum()), int(nbB.sum())
    B = BA + BB
    baseA = np.zeros(tiles, np.int64)
    baseA[1:] = np.cumsum(nbA)[:-1]
    baseB = np.zeros(tiles, np.int64)
    baseB[1:] = np.cumsum(nbB)[:-1]
    base = np.zeros(tiles, np.int64)
    base[1:] = np.cumsum(nb)[:-1]

    idxA = np.zeros((nsh, 16, 8 * BA), np.int16)
    idxB = np.zeros((nsh, 16, 8 * BB), np.int16)
    dstloc = np.full((nsh, P, B), -1.0, np.float32)
    for c in range(nsh):
        m = shard == c
        loc_c = local[m]
        order = np.argsort(loc_c, kind="stable")
        loc_c = loc_c[order]
        prow_c = prow[m][order]
        half_c = half[m][order]
        tile_c = loc_c // P
        for t in range(tiles):
            for h, (nbl, bas, idx) in enumerate(
                    ((nbA, baseA, idxA), (nbB, baseB, idxB))):
                sel = (tile_c == t) & (half_c == h)
                k = int(sel.sum())
                if k == 0:
                    continue
                j = np.arange(k)
                rows = prow_c[sel].astype(np.int16)
                # output slot: block j//P, partition j%P ->
                # call-local position i = (j//P)*128 + (j%P) = j
                col = 8 * int(bas[t]) + j // 16
                idx[c, j % 16, col] = rows
                # dstloc at (partition j%P, global block base[t]+h_off+j//P)
                gb = base[t] + (0 if h == 0 else int(nbA[t])) + j // P
                dstloc[c, j % P, gb] = (loc_c[sel] - t * P).astype(np.float32)
    # replicate idx across the 8 groups of 16 partitions
    idxA = np.tile(idxA, (1, 8, 1)).reshape(nsh, P, 8 * BA)
    idxB = np.tile(idxB, (1, 8, 1)).reshape(nsh, P, 8 * BB)
    dstloc_row = np.ascontiguousarray(dstloc.transpose(0, 2, 1)).reshape(
        nsh, B * P)
    return (nbA, nbB, baseA, baseB, base, B, BA, BB,
            idxA, idxB, dstloc, dstloc_row)


def pack_weights(W, al, ar):
    """[W | W@blockdiag(al) | W@blockdiag(ar) | zero-pad] -> [k, REC] f32."""
    W = np.asarray(W, np.float32)
    al = np.asarray(al, np.float32)
    ar = np.asarray(ar, np.float32)
    H, D = al.shape
    k = W.shape[0]
    W3 = W.reshape(k, H, D)
    Wel = np.einsum("khd,hd->kh", W3, al)
    Wer = np.einsum("khd,hd->kh", W3, ar)
    pad = np.zeros((k, REC - 256 - 2 * H), np.float32)
    return np.concatenate([W, Wel, Wer, pad], axis=1)


def build_bass(nsp, in_dim, nbA, nbB, baseA, baseB, base, B, BA, BB, heads):
    """Build the 3-layer SPMD Bass graph (one graph, 8 cores)."""
    from contextlib import ExitStack

    import concourse.bacc as bacc
    import concourse.bass as bass
    import concourse.mybir as mybir
    import concourse.tile as tile
    from concourse.bass import AP
    from concourse.masks import make_identity

    dt = mybir.dt
    f32, bf16, i16 = dt.float32, dt.bfloat16, dt.int16
    AF = mybir.ActivationFunctionType
    tiles = nsp // P
    kdims = [in_dim, 256, 256]
    nb = nbA + nbB

    nc = bacc.Bacc("TRN2", target_bir_lowering=False, debug=False,
                   num_devices=NSH, num_swdge_queues=4)

    h0T = nc.dram_tensor("h0T", [in_dim, nsp], bf16, kind="ExternalInput")
    wps = [nc.dram_tensor(f"wpack{l}", [kdims[l], REC], bf16,
                          kind="ExternalInput") for l in range(3)]
    bias_d = nc.dram_tensor("biases", [3, 256], f32, kind="ExternalInput")
    idxA_d = nc.dram_tensor("idxA", [P, 8 * BA], i16, kind="ExternalInput")
    idxB_d = nc.dram_tensor("idxB", [P, 8 * BB], i16, kind="ExternalInput")
    dstloc_d = nc.dram_tensor("dstloc", [P, B], bf16, kind="ExternalInput")
    dstrow_d = nc.dram_tensor("dstrow", [1, B * P], bf16,
                              kind="ExternalInput")
    out_d = nc.dram_tensor("out", [nsp, 256], f32, kind="ExternalOutput")

    p_slice = [nc.dram_tensor(f"pslice{l}", [nsp, REC], bf16)
               for l in range(3)]
    p_fullA = [nc.dram_tensor(f"pfullA{l}", [HTA, REC], bf16,
                              addr_space="Shared") for l in range(3)]
    p_fullB = [nc.dram_tensor(f"pfullB{l}", [HTB, REC], bf16,
                              addr_space="Shared") for l in range(3)]

    NBH = int(nb.max())
    with tile.TileContext(nc) as tc, ExitStack() as ctx:
        const = ctx.enter_context(tc.tile_pool(name="const", bufs=1))
        psum_pk = ctx.enter_context(
            tc.tile_pool(name="psum_pk", bufs=2, space="PSUM"))
        psum_ms = ctx.enter_context(
            tc.tile_pool(name="psum_ms", bufs=2, space="PSUM"))
        psum_er = ctx.enter_context(
            tc.tile_pool(name="psum_er", bufs=2, space="PSUM"))
        psum_tr = ctx.enter_context(
            tc.tile_pool(name="psum_tr", bufs=2, space="PSUM"))
        gpool = ctx.enter_context(tc.tile_pool(name="gpool", bufs=3))
        selp = ctx.enter_context(tc.tile_pool(name="selp", bufs=3))
        rpool = ctx.enter_context(tc.tile_pool(name="rpool", bufs=3))
        spool = ctx.enter_context(tc.tile_pool(name="spool", bufs=4))
        mpool = ctx.enter_context(tc.tile_pool(name="mpool", bufs=2))

        # constants / persistent state
        iota_i = const.tile([P, P], dt.int32, name="iota_i", tag="iota_i")
        nc.gpsimd.iota(iota_i[:], pattern=[[1, P]], base=0,
                       channel_multiplier=0)
        iota_bf = const.tile([P, P], bf16, name="iota_bf", tag="iota_bf")
        nc.vector.tensor_copy(iota_bf[:], iota_i[:])
        iotac_i = const.tile([P, 1], dt.int32, name="iotac_i", tag="iotac_i")
        nc.gpsimd.iota(iotac_i[:], pattern=[[1, 1]], base=0,
                       channel_multiplier=1)
        iotac_bf = const.tile([P, 1], bf16, name="iotac_bf", tag="iotac_bf")
        nc.vector.tensor_copy(iotac_bf[:], iotac_i[:])
        ident = const.tile([P, P], bf16, name="ident", tag="ident")
        make_identity(nc, ident[:])

        idxA_sb = const.tile([P, 8 * BA], i16, name="idxA", tag="idxA")
        nc.sync.dma_start(idxA_sb[:], idxA_d[:, :])
        idxB_sb = const.tile([P, 8 * BB], i16, name="idxB", tag="idxB")
        nc.sync.dma_start(idxB_sb[:], idxB_d[:, :])
        dstloc_sb = const.tile([P, B], bf16, name="dstloc", tag="dstloc")
        nc.sync.dma_start(dstloc_sb[:], dstloc_d[:, :])

        hT1 = [const.tile([P, nsp], bf16, name=f"h1_{k}", tag=f"h1_{k}")
               for k in range(2)]
        hT2 = [const.tile([P, nsp], bf16, name=f"h2_{k}", tag=f"h2_{k}")
               for k in range(2)]
        h_ins = [None, hT1, hT2]
        h_outs = [hT1, hT2, None]
        hpool = ctx.enter_context(tc.tile_pool(name="hpool", bufs=3))

        b_tiles, er_alls, w_sbs = [], [], []
        for l in range(3):
            kch = kdims[l] // P
            bt = const.tile([P, 256], f32, name=f"btile{l}", tag=f"btile{l}")
            nc.sync.dma_start(bt[:], bias_d[l:l + 1, :].to_broadcast((P, 256)))
            b_tiles.append(bt)
            er_alls.append(const.tile([P, tiles * heads[l]], bf16,
                                      name=f"erall{l}", tag=f"erall{l}"))
            ws = [const.tile([P, REC], bf16, name=f"w{l}_{k}",
                             tag=f"w{l}_{k}") for k in range(kch)]
            for k in range(kch):
                nc.sync.dma_start(ws[k][:], wps[l][k * P:(k + 1) * P, :])
            w_sbs.append(ws)

        def emit_pack(l, t):
            H = heads[l]
            kch = kdims[l] // P
            if l == 0:
                # stream the (transposed) input features tile from DRAM
                lhs = []
                for k in range(kch):
                    hk = hpool.tile([P, P], bf16, name=f"hin{k}",
                                    tag=f"hin{k}")
                    nc.sync.dma_start(
                        hk[:], h0T[k * P:(k + 1) * P, t * P:(t + 1) * P])
                    lhs.append(hk[:])
            else:
                lhs = [h_ins[l][k][:, t * P:(t + 1) * P]
                       for k in range(kch)]
            ps = psum_pk.tile([P, REC], f32, name="pspk", tag="pspk")
            for k in range(kch):
                nc.tensor.matmul(
                    ps[:], lhsT=lhs[k],
                    rhs=w_sbs[l][k][:], start=(k == 0), stop=(k == kch - 1))
            pack = gpool.tile([P, REC], bf16, name="pack", tag="pack")
            nc.vector.tensor_copy(pack[:, 0:256], ps[:, 0:256])
            pf = pack[:].bitcast(f32)
            el_dst = AP(pf.tensor, pf.offset + ELF, [pf.ap[0], [1, 4]])
            nc.vector.tensor_copy(el_dst, ps[:, 256:260])
            nc.vector.tensor_copy(er_alls[l][:, t * H:(t + 1) * H],
                                  ps[:, 256 + H:256 + 2 * H])
            nc.sync.dma_start(p_slice[l][t * P:(t + 1) * P, :], pack[:])

        def emit_ag(l, hh):
            src_ap = (p_slice[l][0:HLOC, :] if hh == 0
                      else p_slice[l][HLOC:nsp, :])
            out = p_fullA[l] if hh == 0 else p_fullB[l]
            nc.gpsimd.collective_compute(
                "AllGather", mybir.AluOpType.bypass,
                replica_groups=[list(range(NSH))],
                ins=[src_ap.opt()], outs=[out.ap().opt()])

        HTILE = HLOC // P          # 25: tiles covered by table half A
        # layer 0 pack runs standalone (inputs only); allgather in halves
        for t in range(tiles):
            emit_pack(0, t)
            if t == HTILE - 1:
                emit_ag(0, 0)
        emit_ag(0, 1)

        for l in range(3):
            H = heads[l]
            DH = 256 // H
            CH = 256 + H           # scatter-matmul rhs cols: [sum | msg]
            h_out = h_outs[l]
            b_tile = b_tiles[l]
            er_all = er_alls[l]

            # ---- edge phase ----
            qn = [0]
            for t in range(tiles):
                nblk = int(nb[t])
                nba, nbb = int(nbA[t]), int(nbB[t])
                g0 = int(base[t])
                ps_m = psum_ms.tile([P, CH], f32, name="psms", tag="psms")

                # gather source records: dma_gather per table half, chunked
                # to <=1024 indices per call (HW ucode limit)
                CMAX = 8
                G = gpool.tile([P, NBH * REC], bf16, name="G", tag="G")
                g3 = G[:, 0:nblk * REC].rearrange("p (b r) -> p b r", r=REC)
                for cnt, bcol, off, idx_sb, tab in (
                        (nba, int(baseA[t]), 0, idxA_sb, p_fullA[l]),
                        (nbb, int(baseB[t]), nba, idxB_sb, p_fullB[l])):
                    done = 0
                    while done < cnt:
                        c = min(CMAX, cnt - done)
                        nc.gpsimd.dma_gather(
                            out_ap=g3[:, off + done:off + done + c, :],
                            in_ap=tab[:, :],
                            idxs_ap=idx_sb[:, 8 * (bcol + done):
                                           8 * (bcol + done + c)],
                            num_idxs=c * P, num_idxs_reg=c * P,
                            elem_size=REC, queue_num=qn[0] % 4)
                        qn[0] += 1
                        done += c

                # er broadcast: replicate dstloc row, compare vs column iota,
                # then one Nf=H matmul per block against this tile's er rows
                rep = rpool.tile([P, NBH * P], bf16, name="rep", tag="rep")
                nc.sync.dma_start(
                    rep[:, 0:nblk * P],
                    dstrow_d[0:1, g0 * P:(g0 + nblk) * P].to_broadcast(
                        (P, nblk * P)))
                msel = selp.tile([P, NBH * P], bf16, name="msel", tag="msel")
                ioc = iotac_bf[:]
                in1c = AP(ioc.tensor, ioc.offset, [ioc.ap[0], [0, nblk * P]])
                nc.vector.tensor_tensor(out=msel[:, 0:nblk * P],
                                        in0=rep[:, 0:nblk * P], in1=in1c,
                                        op=mybir.AluOpType.is_equal)
                ps_er = psum_er.tile([P, NBH * H], f32, name="pser",
                                     tag="pser")
                for j in range(nblk):
                    nc.tensor.matmul(
                        ps_er[:, j * H:(j + 1) * H],
                        lhsT=msel[:, j * P:(j + 1) * P],
                        rhs=er_all[:, t * H:(t + 1) * H],
                        start=True, stop=True)

                # e = lrelu(el + er); t = exp(e)
                gap = G[:]
                gf = gap.bitcast(f32)
                el_ap = AP(gf.tensor, gf.offset + ELF,
                           [gf.ap[0], [RECF, nblk], [1, H]])
                er3 = ps_er[:, 0:nblk * H].rearrange("p (b h) -> p b h", h=H)
                e1 = spool.tile([P, NBH * H], f32, name="e1", tag="e1")
                e13 = e1[:, 0:nblk * H].rearrange("p (b h) -> p b h", h=H)
                nc.vector.tensor_tensor(out=e13, in0=el_ap, in1=er3,
                                        op=mybir.AluOpType.add)
                e2 = spool.tile([P, NBH * H], f32, name="e2", tag="e2")
                nc.vector.tensor_scalar_mul(
                    e2[:, 0:nblk * H], e1[:, 0:nblk * H], NEG_SLOPE)
                e3 = spool.tile([P, NBH * H], f32, name="e3", tag="e3")
                nc.vector.tensor_tensor(out=e3[:, 0:nblk * H],
                                        in0=e1[:, 0:nblk * H],
                                        in1=e2[:, 0:nblk * H],
                                        op=mybir.AluOpType.max)
                t_bf = spool.tile([P, NBH * H], bf16, name="tbf", tag="tbf")
                nc.scalar.activation(t_bf[:, 0:nblk * H], e3[:, 0:nblk * H],
                                     AF.Exp)

                # scatter selector: dstloc column vs row iota
                sel = selp.tile([P, NBH * P], bf16, name="sel", tag="sel")
                dl = dstloc_sb[:]
                in0 = AP(dl.tensor, dl.offset + g0,
                         [dl.ap[0], [1, nblk], [0, P]])
                io = iota_bf[:]
                in1 = AP(io.tensor, io.offset, [io.ap[0], [0, nblk], [1, P]])
                sel3 = sel[:, 0:nblk * P].rearrange("p (b q) -> p b q", q=P)
                nc.vector.tensor_tensor(out=sel3, in0=in0, in1=in1,
                                        op=mybir.AluOpType.is_equal)

                # rhs = [t | t * feat] per block
                rhs = rpool.tile([P, NBH * CH], bf16, name="rhs", tag="rhs")
                rap = rhs[:]
                t3 = t_bf[:, 0:nblk * H].rearrange("p (b h) -> p b h", h=H)
                s_dst = AP(rap.tensor, rap.offset,
                           [rap.ap[0], [CH, nblk], [1, H]])
                nc.vector.tensor_copy(s_dst, t3)
                gfeat = AP(gap.tensor, gap.offset,
                           [gap.ap[0], [REC, nblk], [DH, H], [1, DH]])
                tb = t_bf[:]
                tmul = AP(tb.tensor, tb.offset,
                          [tb.ap[0], [H, nblk], [1, H], [0, DH]])
                r_dst = AP(rap.tensor, rap.offset + H,
                           [rap.ap[0], [CH, nblk], [DH, H], [1, DH]])
                nc.vector.tensor_tensor(out=r_dst, in0=gfeat, in1=tmul,
                                        op=mybir.AluOpType.mult)

                for j in range(nblk):
                    nc.tensor.matmul(
                        ps_m[:], lhsT=sel[:, j * P:(j + 1) * P],
                        rhs=rhs[:, j * CH:(j + 1) * CH],
                        start=(j == 0), stop=(j == nblk - 1))

                # ---- tile epilogue: msg / sum + bias (+relu, transpose) ----
                s_sb = spool.tile([P, H], f32, name="ssb", tag="ssb")
                nc.vector.tensor_scalar_max(s_sb[:], ps_m[:, 0:H], 1e-30)
                r_sb = spool.tile([P, H], f32, name="rsb", tag="rsb")
                nc.vector.reciprocal(r_sb[:], s_sb[:])
                mn = mpool.tile([P, 256], f32, name="mn", tag="mn")
                mn3 = mn[:].rearrange("p (h d) -> p h d", h=H)
                ms3 = ps_m[:, H:H + 256].rearrange("p (h d) -> p h d", h=H)
                rb = r_sb[:]
                r_bc = AP(rb.tensor, rb.offset, [rb.ap[0], [1, H], [0, DH]])
                nc.vector.tensor_tensor(out=mn3, in0=ms3, in1=r_bc,
                                        op=mybir.AluOpType.mult)
                mb = mpool.tile([P, 256], f32, name="mb", tag="mb")
                nc.vector.tensor_tensor(out=mb[:], in0=mn[:], in1=b_tile[:],
                                        op=mybir.AluOpType.add)
                if l < 2:
                    hb = mpool.tile([P, 256], bf16, name="hb", tag="hb")
                    nc.scalar.activation(hb[:], mb[:], AF.Relu)
                    for k in range(2):
                        pt = psum_tr.tile([P, P], bf16, name="pstr",
                                          tag="pstr")
                        nc.tensor.transpose(pt[:], hb[:, k * P:(k + 1) * P],
                                            ident[:])
                        nc.vector.tensor_copy(
                            h_out[k][:, t * P:(t + 1) * P], pt[:])
                    emit_pack(l + 1, t)
                    if t == HTILE - 1:
                        emit_ag(l + 1, 0)
                else:
                    nc.sync.dma_start(out_d[t * P:(t + 1) * P, :], mb[:])
            if l < 2:
                emit_ag(l + 1, 1)

    nc.compile()
    return nc


def _make_in_maps(feats, wpacks, biases, pre, ns, nsp, in_dim):
    (nbA, nbB, baseA, baseB, base, B, BA, BB,
     idxA, idxB, dstloc, dstloc_row) = pre
    bf = ml_dtypes.bfloat16
    in_maps = []
    for c in range(NSH):
        sl = np.zeros((nsp, in_dim), np.float32)
        sl[:ns] = feats[c * ns:(c + 1) * ns]
        in_maps.append({
            "h0T": np.ascontiguousarray(sl.T).astype(bf),
            "wpack0": wpacks[0].astype(bf),
            "wpack1": wpacks[1].astype(bf),
            "wpack2": wpacks[2].astype(bf),
            "biases": biases.astype(np.float32),
            "idxA": np.ascontiguousarray(idxA[c]),
            "idxB": np.ascontiguousarray(idxB[c]),
            "dstloc": dstloc[c].astype(bf),
            "dstrow": dstloc_row[c].reshape(1, -1).astype(bf),
        })
    return in_maps


def gat_host(feats, src, dst, W0, al0, ar0, b0, W1, al1, ar1, b1,
             W2, al2, ar2, b2, ns=NS, nsp=NSP, in_dim=IN_DIM, run=None):
    """Full host flow: preprocess, build, run (via `run` callback), unshard."""
    feats = np.asarray(feats, np.float32)
    heads = [al0.shape[0], al1.shape[0], al2.shape[0]]
    wpacks = [pack_weights(W0, al0, ar0), pack_weights(W1, al1, ar1),
              pack_weights(W2, al2, ar2)]
    biases = np.stack([np.asarray(b0, np.float32),
                       np.asarray(b1, np.float32),
                       np.asarray(b2, np.float32)])
    pre = preprocess_edges(src, dst, ns, nsp, NSH)
    (nbA, nbB, baseA, baseB, base, B, BA, BB, *_rest) = pre
    nc = build_bass(nsp, in_dim, nbA, nbB, baseA, baseB, base, B, BA, BB,
                    heads)
    in_maps = _make_in_maps(feats, wpacks, biases, pre, ns, nsp, in_dim)
    results = run(nc, in_maps)
    out = np.concatenate([results[c]["out"][:ns] for c in range(NSH)], axis=0)
    return np.ascontiguousarray(out.astype(np.float32))


def kernel(**inputs):
    from concourse.bass_utils import run_bass_kernel_spmd

    trace = os.environ.get("GAT_TRACE", "0") == "1"
    tmpdir = os.environ.get("GAT_TRACE_DIR") or None

    def run(nc, in_maps):
        res = run_bass_kernel_spmd(nc, in_maps, core_ids=list(range(NSH)),
                                   trace=trace, tmpdir=tmpdir)
        if trace:
            print(f"HW exec time: {res.exec_time_ns} ns")
        return res.results

    return gat_host(
        inputs["feats"], inputs["src"], inputs["dst"],
        inputs["W0"], inputs["al0"], inputs["ar0"], inputs["b0"],
        inputs["W1"], inputs["al1"], inputs["ar1"], inputs["b1"],
        inputs["W2"], inputs["al2"], inputs["ar2"], inputs["b2"],
        run=run)
